# revision 2
# baseline (speedup 1.0000x reference)
"""MultiHeadClassifier (MoE routing) Trainium2 kernel.

Problem: B=65536 samples of dim D=1024, each routed by task_id to one of
T=16 two-layer heads (D->H=128 relu -> C=10). The dense reference computes
all 16 heads for every sample (275 GFLOP); here we route on the host and
compute only each sample's own head (~17 GFLOP), data-parallel with 2 tasks
per NeuronCore across 8 cores.

Per-core budget (measured): x-stream DMA ~17.3MB bf16 at ~320-360 GB/s is
the roofline (~50us); PE time (bf16 L1 8 matmuls/512-subtile + L2) is just
under it. Design goals, from baseline trace analysis:
  - One flat DRAM buffer per core laid out in exact consumption order;
    x arrives in ~12 block DMAs (0.25-2MB) on the sync HWDGE ring instead
    of 48 chunk DMAs -> kills per-DMA SDMA-engine boundary bubbles.
  - Tapered block sizes (512 head, 1024 middle, 512/256/128 tail): PE can
    start early and the post-stream tail is tiny.
  - Weights packed to 2 DMAs/slot on the scalar HWDGE ring (lands first);
    outputs per block on gpsimd SWDGE (own queue rows, never blocks x).
  - Short PE warmup on the weight tile itself (no memset), timed to end
    as the first x block lands, so HAM is at K=8/8 for all real matmuls.
  - Fewer tiles/DMAs/instructions -> fewer Tile semaphores -> shorter
    kernel-tail drain+barrier butterfly (it waits per allocated sem).
"""

import sys

import numpy as np

for _p in ("/opt/trn_rl_repo", "/root/.axon_site/_ro/trn_rl_repo"):
    if _p not in sys.path:
        sys.path.append(_p)

import concourse.bacc as bacc
import concourse.mybir as mybir
from concourse.bass_utils import run_bass_kernel_spmd
from concourse.tile import TileContext

B, D, T, H, C = 65536, 1024, 16, 128, 10
N_CORES = 8
S = T // N_CORES  # task slots per core = 2
DC = D // 128  # d-chunks of 128 = 8
MT = 512  # m-subtile (max fp32 PSUM free dim)
WCOLS = DC * H + 16  # w1 (1024) + w2 (10) + pad

MM_DTYPE = "bf16"

_F32 = mybir.dt.float32
_BF16 = mybir.dt.bfloat16


def _np_bf16():
    import ml_dtypes

    return np.dtype(ml_dtypes.bfloat16)


def _plan_blocks(M):
    """Tapered block sizes summing to M: small head (PE starts early),
    1024-col middle, descending tail (short post-stream critical path)."""
    assert M % 32 == 0
    if M <= 1024:
        return [M]
    head, tail = [512], [512, 256, 128]
    rem = M - sum(head) - sum(tail)
    if rem < 0:
        head, tail = [], [512, 256, 128]
        rem = M - sum(tail)
        if rem < 0:
            return [M - 256, 256] if M > 512 else [M]
    mids = []
    while rem > 0:
        c = min(1024, rem)
        if rem - c and rem - c < 256:
            c = rem - 256
        mids.append(c)
        rem -= c
    return head + mids + tail


def _chunks(total, step):
    out = []
    p = 0
    while p < total:
        c = min(step, total - p)
        out.append((p, c))
        p += c
    return out


def _build(M_task, mm_dtype=MM_DTYPE):
    assert mm_dtype == "bf16"
    blocks = _plan_blocks(M_task)
    # interleaved issue order: (b0,s0), (b0,s1), (b1,s0), ...
    sched = [(bi, s) for bi in range(len(blocks)) for s in range(S)]
    # flat x layout: per (block, slot) region [128, DC*xl], partition-major
    offs = {}
    off = 0
    starts = np.concatenate([[0], np.cumsum(blocks)]).astype(int)
    for bi, s in sched:
        xl = blocks[bi]
        offs[(bi, s)] = off
        off += 128 * DC * xl
    total_x = off

    nc = bacc.Bacc(None, target_bir_lowering=False)
    xL = nc.declare_dram_parameter("xL", [total_x], _BF16, isOutput=False)
    wcat = nc.declare_dram_parameter("wcat", [S, 128, WCOLS], _BF16, isOutput=False)
    bcat = nc.declare_dram_parameter("bcat", [S, 128, 2], _F32, isOutput=False)
    outT = nc.declare_dram_parameter("outT", [S, C, M_task], _F32, isOutput=True)

    relu = mybir.ActivationFunctionType.Relu
    N_WARMUP = 6

    with TileContext(nc) as tc:
        with (
            tc.tile_pool(name="wpool", bufs=1) as wpool,
            tc.tile_pool(name="xpool", bufs=1) as xpool,
            tc.tile_pool(name="hpool", bufs=6) as hpool,
            tc.tile_pool(name="opool", bufs=2) as opool,
            tc.tile_pool(name="psum1", bufs=4, space="PSUM") as psum1,
            tc.tile_pool(name="psum2", bufs=3, space="PSUM") as psum2,
            tc.tile_pool(name="psumw", bufs=1, space="PSUM") as psumw,
        ):
            # weights first, on the scalar HWDGE ring (x never queues here)
            wts = []
            for s in range(S):
                wt = wpool.tile([128, WCOLS], _BF16, tag=f"w{s}", name=f"wt{s}")
                nc.scalar.dma_start(wt, wcat[s])
                bt = wpool.tile([128, 2], _F32, tag=f"b{s}", name=f"bt{s}")
                nc.scalar.dma_start(bt, bcat[s])
                wts.append((wt, bt))

            # all x block DMAs up-front on the sync HWDGE ring, in consumption
            # order: the ring FIFO delivers blocks sequentially at line rate
            xts = {}
            for bi, s in sched:
                xl = blocks[bi]
                o = offs[(bi, s)]
                xt = xpool.tile(
                    [128, DC * xl], _BF16, tag=f"x{bi}_{s}", name=f"x{bi}_{s}"
                )
                nc.sync.dma_start(
                    xt, xL[o : o + 128 * DC * xl].rearrange("(p k) -> p k", p=128)
                )
                xts[(bi, s)] = xt

            # PE warmup on slot-0 weights (lands ~8.2us; first x block ~12us):
            # garbage matmuls into a scratch bank release the HAM clock gate
            wps = psumw.tile([128, MT], _F32, tag="wps")
            w0 = wts[0][0]
            for _ in range(N_WARMUP):
                nc.tensor.matmul(wps[:], w0[:, :128], w0[:, :MT], start=True, stop=True)

            for bi, s in sched:
                xl = blocks[bi]
                x0 = starts[bi]
                wt, bt = wts[s]
                xt = xts[(bi, s)]
                b1t = bt[:, 0:1]
                b2t = bt[0:C, 1:2]
                ot = opool.tile([C, xl], _F32, tag=f"o{bi}", name=f"o{bi}_{s}")
                subs = _chunks(xl, MT)
                # waves of <=4 m-subtiles; dc-outer within a wave so the
                # stationary W1 chunk is reused across the wave's matmuls
                for w0i in range(0, len(subs), 4):
                    wave = subs[w0i : w0i + 4]
                    ps1s = [
                        psum1.tile([H, MT], _F32, tag="ps1", name=f"ps1_{j}")
                        for j in range(len(wave))
                    ]
                    for dc in range(DC):
                        lhs = wt[:, dc * H : (dc + 1) * H]
                        for j, (m0, mt) in enumerate(wave):
                            nc.tensor.matmul(
                                ps1s[j][:, :mt],
                                lhs,
                                xt[:, dc * xl + m0 : dc * xl + m0 + mt],
                                start=(dc == 0),
                                stop=(dc == DC - 1),
                            )
                    for j, (m0, mt) in enumerate(wave):
                        ht = hpool.tile([H, MT], _BF16, tag="h")
                        nc.scalar.activation(
                            ht[:, :mt], ps1s[j][:, :mt], relu, bias=b1t
                        )
                        ps2 = psum2.tile([C, MT], _F32, tag="ps2")
                        nc.tensor.matmul(
                            ps2[:, :mt],
                            wt[:, DC * H : DC * H + C],
                            ht[:, :mt],
                            start=True,
                            stop=True,
                        )
                        nc.vector.tensor_tensor(
                            ot[:, m0 : m0 + mt],
                            ps2[:, :mt],
                            b2t.to_broadcast([C, mt]),
                            mybir.AluOpType.add,
                        )
                # gpsimd SWDGE: outputs never head-of-line block the x stream
                nc.gpsimd.dma_start(outT[s, :, x0 : x0 + xl], ot[:])
    nc.compile()
    return nc


def _prepare(x, task_id, W1, b1, W2, b2, mm_dtype=MM_DTYPE):
    """Host-side routing: returns (in_maps, idx, counts, M_task)."""
    bf16 = _np_bf16()
    x = np.ascontiguousarray(np.asarray(x, dtype=np.float32))
    task_id = np.asarray(task_id).astype(np.int64)
    W1 = np.asarray(W1, dtype=np.float32)
    b1 = np.asarray(b1, dtype=np.float32)
    W2 = np.asarray(W2, dtype=np.float32)
    b2 = np.asarray(b2, dtype=np.float32)

    order = np.argsort(task_id, kind="stable")
    counts = np.bincount(task_id, minlength=T)
    starts_t = np.concatenate([[0], np.cumsum(counts)])
    M_task = max(128, int(-(-int(counts.max()) // 32) * 32))

    blocks = _plan_blocks(M_task)
    sched = [(bi, s) for bi in range(len(blocks)) for s in range(S)]
    bstarts = np.concatenate([[0], np.cumsum(blocks)]).astype(int)

    # idx[t] = sample rows for task t, padded with row 0 (discarded later)
    idx = np.zeros((T, M_task), dtype=np.int64)
    for t in range(T):
        idx[t, : counts[t]] = order[starts_t[t] : starts_t[t + 1]]

    in_maps = []
    for c in range(N_CORES):
        ts_c = [S * c + s for s in range(S)]
        # xT[s] = [DC, 128, M] (d-major within chunk on axis 1)
        xTs = []
        for s in range(S):
            xg = x[idx[ts_c[s]]].astype(bf16)  # [M, D]
            xTs.append(np.ascontiguousarray(xg.T).reshape(DC, 128, M_task))
        xL = np.empty(sum(128 * DC * b for b in blocks) * S, dtype=bf16)
        off = 0
        for bi, s in sched:
            xl = blocks[bi]
            x0 = bstarts[bi]
            # region [128, DC, xl] partition-major
            reg = xTs[s][:, :, x0 : x0 + xl].transpose(1, 0, 2)
            n = 128 * DC * xl
            xL[off : off + n] = reg.reshape(-1)
            off += n

        wcat = np.zeros((S, 128, WCOLS), dtype=bf16)
        bcat = np.zeros((S, 128, 2), dtype=np.float32)
        for s in range(S):
            t = ts_c[s]
            # w1 [D,H] -> [128, DC*H] partition-major
            wcat[s, :, : DC * H] = (
                W1[t].reshape(DC, 128, H).transpose(1, 0, 2).reshape(128, DC * H)
            ).astype(bf16)
            wcat[s, :, DC * H : DC * H + C] = W2[t].astype(bf16)
            bcat[s, :, 0] = b1[t]
            bcat[s, :C, 1] = b2[t]

        in_maps.append({"xL": xL, "wcat": wcat, "bcat": bcat})
    return in_maps, idx, counts, M_task


def _unshard(results, idx, counts, b_total=B):
    out = np.empty((b_total, C), dtype=np.float32)
    for c in range(N_CORES):
        yT = np.asarray(results[c]["outT"])  # [S, C, M_task]
        y = yT.transpose(0, 2, 1)  # [S, M_task, C]
        for s in range(S):
            t = S * c + s
            cnt = counts[t]
            out[idx[t, :cnt]] = y[s, :cnt]
    return out


def kernel(x, task_id, W1, b1, W2, b2):
    in_maps, idx, counts, M_task = _prepare(x, task_id, W1, b1, W2, b2)
    nc = _build(M_task)
    try:
        res = run_bass_kernel_spmd(nc, in_maps, list(range(N_CORES)))
    except Exception:
        # transient NRT device hiccups (e.g. NRT_EXEC_UNIT_UNRECOVERABLE)
        # have been observed to succeed on retry
        res = run_bass_kernel_spmd(nc, in_maps, list(range(N_CORES)))
    return _unshard(res.results, idx, counts, b_total=np.asarray(task_id).shape[0])


# revision 4
# speedup vs baseline: 1.0495x; 1.0495x over previous
"""MultiHeadClassifier (MoE routing) Trainium2 kernel.

Problem: B=65536 samples of dim D=1024, each routed by task_id to one of
T=16 two-layer heads (D->H=128 relu -> C=10). The dense reference computes
all 16 heads for every sample (275 GFLOP); here we route on the host and
compute only each sample's own head (~17 GFLOP), data-parallel with 2 tasks
per NeuronCore across 8 cores.

Per-core budget (measured): x-stream DMA ~17.3MB bf16 at ~320-360 GB/s is
the roofline (~50us); PE time (bf16 L1 8 matmuls/512-subtile + L2) is just
under it. Design goals, from baseline trace analysis:
  - One flat DRAM buffer per core laid out in exact consumption order;
    x arrives in ~12 block DMAs (0.25-2MB) on the sync HWDGE ring instead
    of 48 chunk DMAs -> kills per-DMA SDMA-engine boundary bubbles.
  - Tapered block sizes (512 head, 1024 middle, 512/256/128 tail): PE can
    start early and the post-stream tail is tiny.
  - Weights packed to 2 DMAs/slot on the scalar HWDGE ring (lands first);
    outputs per block on gpsimd SWDGE (own queue rows, never blocks x).
  - Short PE warmup on the weight tile itself (no memset), timed to end
    as the first x block lands, so HAM is at K=8/8 for all real matmuls.
  - Fewer tiles/DMAs/instructions -> fewer Tile semaphores -> shorter
    kernel-tail drain+barrier butterfly (it waits per allocated sem).
"""

import sys

import numpy as np

for _p in ("/opt/trn_rl_repo", "/root/.axon_site/_ro/trn_rl_repo"):
    if _p not in sys.path:
        sys.path.append(_p)

import concourse.bacc as bacc
import concourse.mybir as mybir
from concourse.bass_utils import run_bass_kernel_spmd
from concourse.tile import TileContext

B, D, T, H, C = 65536, 1024, 16, 128, 10
N_CORES = 8
S = T // N_CORES  # task slots per core = 2
DC = D // 128  # d-chunks of 128 = 8
MT = 512  # m-subtile (max fp32 PSUM free dim)
WCOLS = DC * H + 16  # w1 (1024) + w2 (10) + pad

MM_DTYPE = "bf16"

_F32 = mybir.dt.float32
_BF16 = mybir.dt.bfloat16


def _np_bf16():
    import ml_dtypes

    return np.dtype(ml_dtypes.bfloat16)


def _plan_blocks(M):
    """Tapered block sizes summing to M: small head (PE starts early),
    1024-col middle, descending tail (short post-stream critical path)."""
    assert M % 32 == 0
    if M <= 1024:
        return [M]
    head, tail = [512], [512, 256, 128]
    rem = M - sum(head) - sum(tail)
    if rem < 0:
        head, tail = [], [512, 256, 128]
        rem = M - sum(tail)
        if rem < 0:
            return [M - 256, 256] if M > 512 else [M]
    mids = []
    while rem > 0:
        c = min(1024, rem)
        if rem - c and rem - c < 256:
            c = rem - 256
        mids.append(c)
        rem -= c
    return head + mids + tail


def _chunks(total, step):
    out = []
    p = 0
    while p < total:
        c = min(step, total - p)
        out.append((p, c))
        p += c
    return out


def _build(M_task, mm_dtype=MM_DTYPE):
    assert mm_dtype == "bf16"
    blocks = _plan_blocks(M_task)
    # interleaved issue order: (b0,s0), (b0,s1), (b1,s0), ...
    sched = [(bi, s) for bi in range(len(blocks)) for s in range(S)]
    # flat x layout: per (block, slot) region [128, DC*xl], partition-major
    offs = {}
    off = 0
    starts = np.concatenate([[0], np.cumsum(blocks)]).astype(int)
    for bi, s in sched:
        xl = blocks[bi]
        offs[(bi, s)] = off
        off += 128 * DC * xl
    total_x = off

    nc = bacc.Bacc(None, target_bir_lowering=False)
    xL = nc.declare_dram_parameter("xL", [total_x], _BF16, isOutput=False)
    wcat = nc.declare_dram_parameter("wcat", [S, 128, WCOLS], _BF16, isOutput=False)
    bcat = nc.declare_dram_parameter("bcat", [S, 128, 2], _F32, isOutput=False)
    outT = nc.declare_dram_parameter("outT", [S, C, M_task], _F32, isOutput=True)

    relu = mybir.ActivationFunctionType.Relu
    N_WARMUP = 6

    with TileContext(nc) as tc:
        with (
            tc.tile_pool(name="wpool", bufs=1) as wpool,
            tc.tile_pool(name="xpool", bufs=1) as xpool,
            tc.tile_pool(name="hpool", bufs=8) as hpool,
            tc.tile_pool(name="opool", bufs=2) as opool,
            tc.tile_pool(name="psum1", bufs=5, space="PSUM") as psum1,
            tc.tile_pool(name="psum2", bufs=2, space="PSUM") as psum2,
            tc.tile_pool(name="psumw", bufs=1, space="PSUM") as psumw,
        ):
            # weights first, on gpsimd SWDGE (x never queues behind them and
            # they land early enough to gate the PE warmup at ~9us)
            wts = []
            for s in range(S):
                wt = wpool.tile([128, WCOLS], _BF16, tag=f"w{s}", name=f"wt{s}")
                nc.gpsimd.dma_start(wt, wcat[s])
                bt = wpool.tile([128, 2], _F32, tag=f"b{s}", name=f"bt{s}")
                nc.gpsimd.dma_start(bt, bcat[s])
                wts.append((wt, bt))

            # all x block DMAs up-front on the sync HWDGE ring, in consumption
            # order: the ring FIFO delivers blocks sequentially at line rate
            xts = {}
            for bi, s in sched:
                xl = blocks[bi]
                o = offs[(bi, s)]
                xt = xpool.tile(
                    [128, DC * xl], _BF16, tag=f"x{bi}_{s}", name=f"x{bi}_{s}"
                )
                nc.sync.dma_start(
                    xt, xL[o : o + 128 * DC * xl].rearrange("(p k) -> p k", p=128)
                )
                xts[(bi, s)] = xt

            # PE warmup on slot-0 weights (lands ~8.2us; first x block ~12us):
            # garbage matmuls into a scratch bank release the HAM clock gate
            wps = psumw.tile([128, MT], _F32, tag="wps")
            w0 = wts[0][0]
            for _ in range(N_WARMUP):
                nc.tensor.matmul(wps[:], w0[:, :128], w0[:, :MT], start=True, stop=True)

            for bi, s in sched:
                xl = blocks[bi]
                x0 = starts[bi]
                wt, bt = wts[s]
                xt = xts[(bi, s)]
                b1t = bt[:, 0:1]
                b2t = bt[0:C, 1:2]
                ot = opool.tile([C, xl], _F32, tag=f"o{bi}", name=f"o{bi}_{s}")
                subs = _chunks(xl, MT)
                # waves of <=4 m-subtiles; dc-outer within a wave so the
                # stationary W1 chunk is reused across the wave's matmuls
                for w0i in range(0, len(subs), 4):
                    wave = subs[w0i : w0i + 4]
                    ps1s = [
                        psum1.tile([H, MT], _F32, tag="ps1", name=f"ps1_{j}")
                        for j in range(len(wave))
                    ]
                    for dc in range(DC):
                        lhs = wt[:, dc * H : (dc + 1) * H]
                        for j, (m0, mt) in enumerate(wave):
                            nc.tensor.matmul(
                                ps1s[j][:, :mt],
                                lhs,
                                xt[:, dc * xl + m0 : dc * xl + m0 + mt],
                                start=(dc == 0),
                                stop=(dc == DC - 1),
                            )
                    # relu+b1 for the whole wave first, then the wave's L2
                    # matmuls back-to-back into disjoint 32-col strips of the
                    # PE array (col tiling): they run concurrently, ~1 matmul
                    # cost for up to 4
                    hts = []
                    for j, (m0, mt) in enumerate(wave):
                        ht = hpool.tile([H, MT], _BF16, tag="h")
                        nc.scalar.activation(
                            ht[:, :mt], ps1s[j][:, :mt], relu, bias=b1t
                        )
                        hts.append(ht)
                    ps2 = psum2.tile([128, MT], _F32, tag="ps2")
                    for j, (m0, mt) in enumerate(wave):
                        nc.tensor.matmul(
                            ps2[32 * j : 32 * j + C, :mt],
                            wt[:, DC * H : DC * H + C],
                            hts[j][:, :mt],
                            start=True,
                            stop=True,
                            tile_position=(0, 32 * j),
                        )
                    for j, (m0, mt) in enumerate(wave):
                        nc.vector.tensor_tensor(
                            ot[:, m0 : m0 + mt],
                            ps2[32 * j : 32 * j + C, :mt],
                            b2t.to_broadcast([C, mt]),
                            mybir.AluOpType.add,
                        )
                # gpsimd SWDGE: outputs never head-of-line block the x stream
                nc.gpsimd.dma_start(outT[s, :, x0 : x0 + xl], ot[:])
    nc.compile()
    return nc


def _prepare(x, task_id, W1, b1, W2, b2, mm_dtype=MM_DTYPE):
    """Host-side routing: returns (in_maps, idx, counts, M_task)."""
    bf16 = _np_bf16()
    x = np.ascontiguousarray(np.asarray(x, dtype=np.float32))
    task_id = np.asarray(task_id).astype(np.int64)
    W1 = np.asarray(W1, dtype=np.float32)
    b1 = np.asarray(b1, dtype=np.float32)
    W2 = np.asarray(W2, dtype=np.float32)
    b2 = np.asarray(b2, dtype=np.float32)

    order = np.argsort(task_id, kind="stable")
    counts = np.bincount(task_id, minlength=T)
    starts_t = np.concatenate([[0], np.cumsum(counts)])
    M_task = max(128, int(-(-int(counts.max()) // 32) * 32))

    blocks = _plan_blocks(M_task)
    sched = [(bi, s) for bi in range(len(blocks)) for s in range(S)]
    bstarts = np.concatenate([[0], np.cumsum(blocks)]).astype(int)

    # idx[t] = sample rows for task t, padded with row 0 (discarded later)
    idx = np.zeros((T, M_task), dtype=np.int64)
    for t in range(T):
        idx[t, : counts[t]] = order[starts_t[t] : starts_t[t + 1]]

    in_maps = []
    for c in range(N_CORES):
        ts_c = [S * c + s for s in range(S)]
        # xT[s] = [DC, 128, M] (d-major within chunk on axis 1)
        xTs = []
        for s in range(S):
            xg = x[idx[ts_c[s]]].astype(bf16)  # [M, D]
            xTs.append(np.ascontiguousarray(xg.T).reshape(DC, 128, M_task))
        xL = np.empty(sum(128 * DC * b for b in blocks) * S, dtype=bf16)
        off = 0
        for bi, s in sched:
            xl = blocks[bi]
            x0 = bstarts[bi]
            # region [128, DC, xl] partition-major
            reg = xTs[s][:, :, x0 : x0 + xl].transpose(1, 0, 2)
            n = 128 * DC * xl
            xL[off : off + n] = reg.reshape(-1)
            off += n

        wcat = np.zeros((S, 128, WCOLS), dtype=bf16)
        bcat = np.zeros((S, 128, 2), dtype=np.float32)
        for s in range(S):
            t = ts_c[s]
            # w1 [D,H] -> [128, DC*H] partition-major
            wcat[s, :, : DC * H] = (
                W1[t].reshape(DC, 128, H).transpose(1, 0, 2).reshape(128, DC * H)
            ).astype(bf16)
            wcat[s, :, DC * H : DC * H + C] = W2[t].astype(bf16)
            bcat[s, :, 0] = b1[t]
            bcat[s, :C, 1] = b2[t]

        in_maps.append({"xL": xL, "wcat": wcat, "bcat": bcat})
    return in_maps, idx, counts, M_task


def _unshard(results, idx, counts, b_total=B):
    out = np.empty((b_total, C), dtype=np.float32)
    for c in range(N_CORES):
        yT = np.asarray(results[c]["outT"])  # [S, C, M_task]
        y = yT.transpose(0, 2, 1)  # [S, M_task, C]
        for s in range(S):
            t = S * c + s
            cnt = counts[t]
            out[idx[t, :cnt]] = y[s, :cnt]
    return out


def kernel(x, task_id, W1, b1, W2, b2):
    in_maps, idx, counts, M_task = _prepare(x, task_id, W1, b1, W2, b2)
    nc = _build(M_task)
    try:
        res = run_bass_kernel_spmd(nc, in_maps, list(range(N_CORES)))
    except Exception:
        # transient NRT device hiccups (e.g. NRT_EXEC_UNIT_UNRECOVERABLE)
        # have been observed to succeed on retry
        res = run_bass_kernel_spmd(nc, in_maps, list(range(N_CORES)))
    return _unshard(res.results, idx, counts, b_total=np.asarray(task_id).shape[0])


# revision 5
# speedup vs baseline: 1.0836x; 1.0325x over previous
"""MultiHeadClassifier (MoE routing) Trainium2 kernel.

Problem: B=65536 samples of dim D=1024, each routed by task_id to one of
T=16 two-layer heads (D->H=128 relu -> C=10). The dense reference computes
all 16 heads for every sample (275 GFLOP); here we route on the host and
compute only each sample's own head (~17 GFLOP), data-parallel with 2 tasks
per NeuronCore across 8 cores.

Per-core budget (measured): x-stream DMA ~17.3MB bf16 at ~320-360 GB/s is
the roofline (~50us); PE time (bf16 L1 8 matmuls/512-subtile + L2) is just
under it. Design goals, from baseline trace analysis:
  - One flat DRAM buffer per core laid out in exact consumption order;
    x arrives in ~12 block DMAs (0.25-2MB) on the sync HWDGE ring instead
    of 48 chunk DMAs -> kills per-DMA SDMA-engine boundary bubbles.
  - Tapered block sizes (512 head, 1024 middle, 512/256/128 tail): PE can
    start early and the post-stream tail is tiny.
  - Weights packed to 2 DMAs/slot on the scalar HWDGE ring (lands first);
    outputs per block on gpsimd SWDGE (own queue rows, never blocks x).
  - Short PE warmup on the weight tile itself (no memset), timed to end
    as the first x block lands, so HAM is at K=8/8 for all real matmuls.
  - Fewer tiles/DMAs/instructions -> fewer Tile semaphores -> shorter
    kernel-tail drain+barrier butterfly (it waits per allocated sem).
"""

import sys

import numpy as np

for _p in ("/opt/trn_rl_repo", "/root/.axon_site/_ro/trn_rl_repo"):
    if _p not in sys.path:
        sys.path.append(_p)

import concourse.bacc as bacc
import concourse.mybir as mybir
from concourse.bass_utils import run_bass_kernel_spmd
from concourse.tile import TileContext

B, D, T, H, C = 65536, 1024, 16, 128, 10
N_CORES = 8
S = T // N_CORES  # task slots per core = 2
DC = D // 128  # d-chunks of 128 = 8
MT = 512  # m-subtile (max fp32 PSUM free dim)
WCOLS = DC * H + 16  # w1 (1024) + w2 (10) + pad

MM_DTYPE = "bf16"

_F32 = mybir.dt.float32
_BF16 = mybir.dt.bfloat16


def _np_bf16():
    import ml_dtypes

    return np.dtype(ml_dtypes.bfloat16)


def _plan_blocks(M):
    """Tapered block sizes summing to M: small head (PE starts early),
    1024-col middle, descending tail (short post-stream critical path)."""
    assert M % 32 == 0
    if M <= 1024:
        return [M]
    head, tail = [512], [512, 256, 128]
    rem = M - sum(head) - sum(tail)
    if rem < 0:
        head, tail = [], [512, 256, 128]
        rem = M - sum(tail)
        if rem < 0:
            return [M - 256, 256] if M > 512 else [M]
    mids = []
    while rem > 0:
        c = min(1024, rem)
        if rem - c and rem - c < 256:
            c = rem - 256
        mids.append(c)
        rem -= c
    return head + mids + tail


def _chunks(total, step):
    out = []
    p = 0
    while p < total:
        c = min(step, total - p)
        out.append((p, c))
        p += c
    return out


def _build(M_task, mm_dtype=MM_DTYPE):
    assert mm_dtype == "bf16"
    blocks = _plan_blocks(M_task)
    # interleaved issue order: (b0,s0), (b0,s1), (b1,s0), ...
    sched = [(bi, s) for bi in range(len(blocks)) for s in range(S)]
    # flat x layout: per (block, slot) region [128, DC*xl], partition-major
    offs = {}
    off = 0
    starts = np.concatenate([[0], np.cumsum(blocks)]).astype(int)
    for bi, s in sched:
        xl = blocks[bi]
        offs[(bi, s)] = off
        off += 128 * DC * xl
    total_x = off

    nc = bacc.Bacc(None, target_bir_lowering=False)
    xL = nc.declare_dram_parameter("xL", [total_x], _BF16, isOutput=False)
    wcat = nc.declare_dram_parameter("wcat", [S, 128, WCOLS], _BF16, isOutput=False)
    bcat = nc.declare_dram_parameter("bcat", [S, 128, 2], _F32, isOutput=False)
    outT = nc.declare_dram_parameter("outT", [S, C, M_task], _F32, isOutput=True)

    relu = mybir.ActivationFunctionType.Relu
    N_WARMUP = 6

    with TileContext(nc) as tc:
        with (
            tc.tile_pool(name="wpool", bufs=1) as wpool,
            tc.tile_pool(name="xpool", bufs=1) as xpool,
            tc.tile_pool(name="hpool", bufs=8) as hpool,
            tc.tile_pool(name="opool", bufs=1) as opool,
            tc.tile_pool(name="psum1", bufs=5, space="PSUM") as psum1,
            tc.tile_pool(name="psum2", bufs=2, space="PSUM") as psum2,
            tc.tile_pool(name="psumw", bufs=1, space="PSUM") as psumw,
        ):
            # weights first on the sync ring (land ~9.5us, gate PE warmup);
            # tiny biases on the scalar ring; gpsimd/SWDGE stays COLD all
            # kernel (its descriptor rings contend with SDMA engines 7/15
            # and make them straggle, serializing the whole x stream)
            wts = []
            for s in range(S):
                wt = wpool.tile([128, WCOLS], _BF16, tag=f"w{s}", name=f"wt{s}")
                nc.sync.dma_start(wt, wcat[s])
                bt = wpool.tile([128, 2], _F32, tag=f"b{s}", name=f"bt{s}")
                nc.scalar.dma_start(bt, bcat[s])
                wts.append((wt, bt))

            # all x block DMAs up-front on the sync HWDGE ring, in consumption
            # order: the ring FIFO delivers blocks sequentially at line rate
            xts = {}
            for bi, s in sched:
                xl = blocks[bi]
                o = offs[(bi, s)]
                xt = xpool.tile(
                    [128, DC * xl], _BF16, tag=f"x{bi}_{s}", name=f"x{bi}_{s}"
                )
                nc.sync.dma_start(
                    xt, xL[o : o + 128 * DC * xl].rearrange("(p k) -> p k", p=128)
                )
                xts[(bi, s)] = xt

            # PE warmup on slot-0 weights (lands ~8.2us; first x block ~12us):
            # garbage matmuls into a scratch bank release the HAM clock gate
            wps = psumw.tile([128, MT], _F32, tag="wps")
            w0 = wts[0][0]
            for _ in range(N_WARMUP):
                nc.tensor.matmul(wps[:], w0[:, :128], w0[:, :MT], start=True, stop=True)

            for bi, s in sched:
                xl = blocks[bi]
                x0 = starts[bi]
                wt, bt = wts[s]
                xt = xts[(bi, s)]
                b1t = bt[:, 0:1]
                b2t = bt[0:C, 1:2]
                ot = opool.tile([C, xl], _F32, tag=f"o{bi}_{s}", name=f"o{bi}_{s}")
                subs = _chunks(xl, MT)
                # waves of <=4 m-subtiles; dc-outer within a wave so the
                # stationary W1 chunk is reused across the wave's matmuls
                for w0i in range(0, len(subs), 4):
                    wave = subs[w0i : w0i + 4]
                    ps1s = [
                        psum1.tile([H, MT], _F32, tag="ps1", name=f"ps1_{j}")
                        for j in range(len(wave))
                    ]
                    for dc in range(DC):
                        lhs = wt[:, dc * H : (dc + 1) * H]
                        for j, (m0, mt) in enumerate(wave):
                            nc.tensor.matmul(
                                ps1s[j][:, :mt],
                                lhs,
                                xt[:, dc * xl + m0 : dc * xl + m0 + mt],
                                start=(dc == 0),
                                stop=(dc == DC - 1),
                            )
                    # relu+b1 for the whole wave first, then the wave's L2
                    # matmuls back-to-back into disjoint 32-col strips of the
                    # PE array (col tiling): they run concurrently, ~1 matmul
                    # cost for up to 4
                    hts = []
                    for j, (m0, mt) in enumerate(wave):
                        ht = hpool.tile([H, MT], _BF16, tag="h")
                        nc.scalar.activation(
                            ht[:, :mt], ps1s[j][:, :mt], relu, bias=b1t
                        )
                        hts.append(ht)
                    ps2 = psum2.tile([128, MT], _F32, tag="ps2")
                    for j, (m0, mt) in enumerate(wave):
                        nc.tensor.matmul(
                            ps2[32 * j : 32 * j + C, :mt],
                            wt[:, DC * H : DC * H + C],
                            hts[j][:, :mt],
                            start=True,
                            stop=True,
                            tile_position=(0, 32 * j),
                        )
                    for j, (m0, mt) in enumerate(wave):
                        nc.vector.tensor_tensor(
                            ot[:, m0 : m0 + mt],
                            ps2[32 * j : 32 * j + C, :mt],
                            b2t.to_broadcast([C, mt]),
                            mybir.AluOpType.add,
                        )
                # sync ring: out descriptors queue behind the x stream and
                # drain right after it; ot tiles are never recycled (distinct
                # tags) so compute never waits on these
                nc.sync.dma_start(outT[s, :, x0 : x0 + xl], ot[:])
    nc.compile()
    return nc


def _prepare(x, task_id, W1, b1, W2, b2, mm_dtype=MM_DTYPE):
    """Host-side routing: returns (in_maps, idx, counts, M_task)."""
    bf16 = _np_bf16()
    x = np.ascontiguousarray(np.asarray(x, dtype=np.float32))
    task_id = np.asarray(task_id).astype(np.int64)
    W1 = np.asarray(W1, dtype=np.float32)
    b1 = np.asarray(b1, dtype=np.float32)
    W2 = np.asarray(W2, dtype=np.float32)
    b2 = np.asarray(b2, dtype=np.float32)

    order = np.argsort(task_id, kind="stable")
    counts = np.bincount(task_id, minlength=T)
    starts_t = np.concatenate([[0], np.cumsum(counts)])
    M_task = max(128, int(-(-int(counts.max()) // 32) * 32))

    blocks = _plan_blocks(M_task)
    sched = [(bi, s) for bi in range(len(blocks)) for s in range(S)]
    bstarts = np.concatenate([[0], np.cumsum(blocks)]).astype(int)

    # idx[t] = sample rows for task t, padded with row 0 (discarded later)
    idx = np.zeros((T, M_task), dtype=np.int64)
    for t in range(T):
        idx[t, : counts[t]] = order[starts_t[t] : starts_t[t + 1]]

    in_maps = []
    for c in range(N_CORES):
        ts_c = [S * c + s for s in range(S)]
        # xT[s] = [DC, 128, M] (d-major within chunk on axis 1)
        xTs = []
        for s in range(S):
            xg = x[idx[ts_c[s]]].astype(bf16)  # [M, D]
            xTs.append(np.ascontiguousarray(xg.T).reshape(DC, 128, M_task))
        xL = np.empty(sum(128 * DC * b for b in blocks) * S, dtype=bf16)
        off = 0
        for bi, s in sched:
            xl = blocks[bi]
            x0 = bstarts[bi]
            # region [128, DC, xl] partition-major
            reg = xTs[s][:, :, x0 : x0 + xl].transpose(1, 0, 2)
            n = 128 * DC * xl
            xL[off : off + n] = reg.reshape(-1)
            off += n

        wcat = np.zeros((S, 128, WCOLS), dtype=bf16)
        bcat = np.zeros((S, 128, 2), dtype=np.float32)
        for s in range(S):
            t = ts_c[s]
            # w1 [D,H] -> [128, DC*H] partition-major
            wcat[s, :, : DC * H] = (
                W1[t].reshape(DC, 128, H).transpose(1, 0, 2).reshape(128, DC * H)
            ).astype(bf16)
            wcat[s, :, DC * H : DC * H + C] = W2[t].astype(bf16)
            bcat[s, :, 0] = b1[t]
            bcat[s, :C, 1] = b2[t]

        in_maps.append({"xL": xL, "wcat": wcat, "bcat": bcat})
    return in_maps, idx, counts, M_task


def _unshard(results, idx, counts, b_total=B):
    out = np.empty((b_total, C), dtype=np.float32)
    for c in range(N_CORES):
        yT = np.asarray(results[c]["outT"])  # [S, C, M_task]
        y = yT.transpose(0, 2, 1)  # [S, M_task, C]
        for s in range(S):
            t = S * c + s
            cnt = counts[t]
            out[idx[t, :cnt]] = y[s, :cnt]
    return out


def kernel(x, task_id, W1, b1, W2, b2):
    in_maps, idx, counts, M_task = _prepare(x, task_id, W1, b1, W2, b2)
    nc = _build(M_task)
    try:
        res = run_bass_kernel_spmd(nc, in_maps, list(range(N_CORES)))
    except Exception:
        # transient NRT device hiccups (e.g. NRT_EXEC_UNIT_UNRECOVERABLE)
        # have been observed to succeed on retry
        res = run_bass_kernel_spmd(nc, in_maps, list(range(N_CORES)))
    return _unshard(res.results, idx, counts, b_total=np.asarray(task_id).shape[0])
